# revision 47
# baseline (speedup 1.0000x reference)
"""TRN2 Bass kernel for nn_DSSMEmbed (vq_codebook).

Strategy (8 NeuronCores, data-parallel over batch, 256 imgs/core):
  - Activation layout: partitions = (x, channel) rows, free = (y, img).
  - 3x3 convs as Toeplitz matmuls over x-windows with batch streamed in N;
    dy handled by PSUM accumulation at shifted free-dim (y) offsets.
  - emb conv: 64x32 8-tile mode, windowed one-hot input from DRAM (K=56).
  - c1 conv:  64x32 8-tile, windowed y-pair buffers built by DMA.
  - c2 conv:  64x64 4-tile, windowed.
  - Tower1 (bf16) and tower2 (fp32, feeds VQ argmax exactly) phases are
    interleaved so each tower's window-build DMA hides under the other
    tower's PE work.
  - VQ: scores.T via PE (fp32), per-row max/max_index on DVE; only the
    ARGMAX INDICES are AllGathered (1KB), then each core builds a one-hot
    gather matrix G[z, n] = (idx_global[n] == z) on DVE and computes
    out = (znT_bf16 . e1norm).T @ G on the PE -- no codebook-row exchange,
    no z transposes.
  - embed1 norms via DVE square+reduce; 1/(|e|+eps) and exp(scale)
    folded into embT1 before its transpose, so the final product needs
    no post-scaling; output DMA'd per 512-column chunk as computed.
"""
import sys

sys.path.insert(0, "/opt/trn_rl_repo")

import numpy as np
import concourse.bass as bass
import concourse.bacc as bacc
import concourse.mybir as mybir
import concourse.tile as tile
from concourse.bass_utils import run_bass_kernel_spmd

F32 = mybir.dt.float32
F16 = mybir.dt.float16
BF16 = mybir.dt.bfloat16
U32 = mybir.dt.uint32
AF = mybir.ActivationFunctionType

NCORES = 8
B = 2048
BL = B // NCORES          # 256 imgs per core
H = W = 16
DICT, SE, CE, ESZ, NZ = 14, 8, 16, 512, 512
EPS = 1e-4
YB = H * BL               # free dim (y, img) = 4096

# ---------------------------------------------------------------------------
# host-side preprocessing
# ---------------------------------------------------------------------------


def make_windowed_oh(nat):
    """nat: (DICT, H, W, Bloc) one-hot -> (4, 4, 128, 6, Bloc).

    px=2: 8 blocks; tensor t holds block t at rows 0.. and block t+4 at
    rows 64..; rows w*14+d for window x' = 2b-1+w, w in 0..3.  Second dim
    is the y-quarter: quarter q covers global y in [4q-1, 4q+5) (clipped,
    duplicated halo) so each DMA load is contiguous per partition.
    """
    out = np.zeros((4, 4, 128, 6, nat.shape[-1]), dtype=np.int8)
    for b in range(8):
        t, h = b % 4, b // 4
        for w in range(4):
            xs = 2 * b - 1 + w
            if 0 <= xs < W:
                for q in range(4):
                    ys, ye = max(0, 4 * q - 1), min(H, 4 * q + 5)
                    out[t, q, h * 64 + w * DICT:h * 64 + (w + 1) * DICT,
                        ys - (4 * q - 1):ye - (4 * q - 1)] = nat[:, ys:ye, xs, :]
    return out


def op_emb_win(wfold):
    """Folded emb conv operator for 64x32 windowed scheme: (3, 4, 128, 32)."""
    op = np.zeros((3, 4, 128, 32), dtype=np.float32)
    for dy in range(3):
        blk = np.zeros((56, 32), dtype=np.float32)
        for w in range(4):
            for xr in range(2):
                dx = w - xr
                if 0 <= dx <= 2:
                    blk[w * DICT:(w + 1) * DICT, xr * 16:(xr + 1) * 16] = \
                        wfold[:, :, dy, dx].T
        for h in range(2):
            op[dy, :, h * 64:h * 64 + 56, :] = blk[None]
    return op


def op_conv_win(wc, c_in, c_out):
    """Windowed 64-row conv operator: (3, 4, 128, px*c_out) with px=2."""
    M = 2 * c_out
    op = np.zeros((3, 4, 128, M), dtype=np.float32)
    blk = np.zeros((4 * c_in, M), dtype=np.float32)
    for dy in range(3):
        blk[:] = 0.0
        for w in range(4):
            for xr in range(2):
                dx = w - xr
                if 0 <= dx <= 2:
                    blk[w * c_in:(w + 1) * c_in, xr * c_out:(xr + 1) * c_out] = \
                        wc[:, :, dy, dx].T
        for h in range(2):
            op[dy, :, h * 64:h * 64 + 4 * c_in, :] = blk[None]
        op[dy, 0, 0:c_in, :] = 0.0                    # b=0, w=0 (x'=-1)
        op[dy, 3, 64 + 3 * c_in:64 + 4 * c_in, :] = 0.0  # b=7, w=3 (x'=16)
    return op


def host_prep(inputs):
    s = np.asarray(inputs["s"])
    sp = np.asarray(inputs["s_prime"])
    se_w = np.asarray(inputs["state_embed"], dtype=np.float32)
    norms = np.sqrt((se_w * se_w).sum(1, keepdims=True))
    table = se_w / np.maximum(norms, 1.0)

    oh_s = (np.arange(DICT)[:, None, None, None] ==
            s.transpose(1, 2, 0)[None]).astype(np.float32)
    oh_sp = (np.arange(DICT)[:, None, None, None] ==
             sp.transpose(1, 2, 0)[None]).astype(np.float32)
    oh_d = oh_sp - oh_s

    emb_fold = np.einsum("oikl,di->odkl",
                         np.asarray(inputs["conv_embed_w"], np.float32), table)

    shared = {
        "op_emb": op_emb_win(emb_fold),
        "op_c1t1": op_conv_win(np.asarray(inputs["p1c1_w"], np.float32), 16, 16),
        "op_c1t2": op_conv_win(np.asarray(inputs["p2c1_w"], np.float32), 16, 16),
        "op_c2t1": op_conv_win(np.asarray(inputs["p1c2_w"], np.float32), 16, 32),
        "op_c2t2": op_conv_win(np.asarray(inputs["p2c2_w"], np.float32), 16, 32),
    }

    def reorder_lin(lw):
        # K order: (chunk c, y, row r), r = xr*32+ch, x = c*4+xr
        lw = np.asarray(lw, np.float32).reshape(ESZ, 32, H, W)
        lw = lw.transpose(3, 1, 2, 0).reshape(4, 4, 32, H, ESZ)  # (c,xr,ch,y,E)
        return np.ascontiguousarray(
            lw.transpose(0, 3, 1, 2, 4).reshape(4, H, 128, ESZ).reshape(64, 128, ESZ))

    shared["lw_t1"] = reorder_lin(inputs["p1l_w"])
    shared["lw_t2"] = reorder_lin(inputs["p2l_w"])

    zv = np.asarray(inputs["z_vectors"], np.float32)
    zn = zv / np.sqrt((zv * zv).sum(1, keepdims=True))
    shared["znT"] = np.ascontiguousarray(zn.T)

    def conv_bias(bvec, c_out):
        reps = 128 // c_out
        return np.ascontiguousarray(
            np.tile(np.asarray(bvec, np.float32), reps)[:, None])

    shared["b_emb"] = conv_bias(inputs["conv_embed_b"], 16)
    shared["b_c1t1"] = conv_bias(inputs["p1c1_b"], 16)
    shared["b_c1t2"] = conv_bias(inputs["p2c1_b"], 16)
    shared["b_c2t1"] = conv_bias(inputs["p1c2_b"], 32)
    shared["b_c2t2"] = conv_bias(inputs["p2c2_b"], 32)
    shared["b_l1"] = np.ascontiguousarray(
        np.asarray(inputs["p1l_b"], np.float32).reshape(1, ESZ))
    shared["b_l2"] = np.ascontiguousarray(
        np.asarray(inputs["p2l_b"], np.float32).reshape(1, ESZ))

    # per-partition iota for the one-hot gather build: iotaz[zc][p] = 128*zc+p
    shared["iotaz"] = np.ascontiguousarray(
        (np.arange(NZ, dtype=np.float32).reshape(4, 128, 1)))

    esc = float(np.exp(np.asarray(inputs["scale"], np.float32).reshape(-1)[0]))

    percore = []
    for c in range(NCORES):
        sl = slice(c * BL, (c + 1) * BL)
        percore.append({
            "ohs": make_windowed_oh(oh_s[..., sl]),
            "ohd": make_windowed_oh(oh_d[..., sl]),
        })
    return shared, percore, esc


# ---------------------------------------------------------------------------
# device program
# ---------------------------------------------------------------------------


def _clip_dy(y0, ny, dy):
    s = max(y0, -dy)
    e = min(y0 + ny, H - dy)
    if s >= e:
        return None
    return (s - y0) * BL, (e - s) * BL, s + dy


def build_program(esc, debug=False):
    from contextlib import ExitStack
    nc = bacc.Bacc("TRN2", target_bir_lowering=False, debug=False,
                   num_devices=NCORES)

    def din(name, shape, dt):
        return nc.dram_tensor(name, list(shape), dt, kind="ExternalInput").ap()

    ohs_d = din("ohs", (4, 4, 128, 6, BL), BF16)
    ohd_d = din("ohd", (4, 4, 128, 6, BL), BF16)
    op_embt1_d = din("op_embt1", (3, 4, 128, 32), BF16)
    op_embt2h_d = din("op_embt2h", (3, 4, 128, 32), BF16)
    op_embt2l_d = din("op_embt2l", (3, 4, 128, 32), BF16)
    op_c1t1_d = din("op_c1t1", (3, 4, 128, 32), BF16)
    op_c1t2_d = din("op_c1t2", (3, 4, 128, 32), F32)
    op_c2t1_d = din("op_c2t1", (3, 4, 128, 64), BF16)
    op_c2t2_d = din("op_c2t2", (3, 4, 128, 64), F32)
    lw1_d = din("lw1", (64, 128, ESZ), BF16)
    lw2h_d = din("lw2h", (64, 128, ESZ), F16)
    lw2l_d = din("lw2l", (64, 128, ESZ), F16)
    b_se_d = din("b_se", (128, 1), F32)
    b_c1t1_d = din("b_c1t1", (128, 1), F32)
    b_c1t2_d = din("b_c1t2", (128, 1), F32)
    b_c2t1_d = din("b_c2t1", (128, 1), F32)
    b_c2t2_d = din("b_c2t2", (128, 1), F32)
    b_l1_d = din("b_l1", (1, ESZ), F32)
    b_l2_d = din("b_l2", (1, ESZ), F32)
    znt_d = din("znt", (ESZ, NZ), F32)
    zntb_d = din("zntb", (ESZ, NZ), BF16)
    iotaz_d = din("iotaz", (4, 128, 1), F32)
    ident_d = din("ident", (128, 128), F32)

    out_d = nc.dram_tensor("out", [BL, B], F32, kind="ExternalOutput").ap()

    iloc_d = nc.dram_tensor("iloc", [BL, 1], F32).ap()
    ig_d = nc.dram_tensor("ig", [NCORES * BL, 1], F32,
                          addr_space="Shared").ap()

    with tile.TileContext(nc) as tc, ExitStack() as ES:
        cst = ES.enter_context(tc.tile_pool(name="cst", bufs=1))
        epool = ES.enter_context(tc.tile_pool(name="emb", bufs=1))
        npool = None

        ident_sb = cst.tile([128, 128], F32, tag="ident", name="ident")
        nc.sync.dma_start(ident_sb[:], ident_d[:])
        bias_sb = {}
        for nm, d in [("b_se", b_se_d), ("b_c1t1", b_c1t1_d),
                      ("b_c1t2", b_c1t2_d), ("b_c2t1", b_c2t1_d),
                      ("b_c2t2", b_c2t2_d)]:
            t = cst.tile([128, 1], F32, tag=nm, name=nm)
            nc.sync.dma_start(t[:], d[:])
            bias_sb[nm] = t
        bl_sb = {}
        for nm, d in [("b_l1", b_l1_d), ("b_l2", b_l2_d)]:
            t = cst.tile([1, ESZ], F32, tag=f"{nm}r", name=f"{nm}r")
            nc.sync.dma_start(t[:], d[:])
            bl_sb[nm] = t
        ones_k = cst.tile([1, 128], F32, tag="ones_k", name="ones_k")
        nc.vector.memset(ones_k[:], 1.0)
        ones_h = cst.tile([1, 128], F16, tag="ones_h", name="ones_h")
        nc.vector.memset(ones_h[:], 1.0)
        znt_sb = []
        for e in range(4):
            t = cst.tile([128, NZ], F32, tag=f"znt{e}", name=f"znt{e}")
            nc.scalar.dma_start(t[:], znt_d[128 * e:128 * e + 128, :])
            znt_sb.append(t)
        iotaz_sb = []
        for zc in range(4):
            t = cst.tile([128, 1], F32, tag=f"iota{zc}", name=f"iota{zc}")
            nc.sync.dma_start(t[:], iotaz_d[zc])
            iotaz_sb.append(t)

        def load_ops(op_d, dt, width, nt, pfx):
            ops = [[cst.tile([128, width], dt, tag=f"{pfx}{dy}{t}",
                             name=f"{pfx}{dy}{t}") for t in range(nt)]
                   for dy in range(3)]
            for dy in range(3):
                for t in range(nt):
                    nc.sync.dma_start(ops[dy][t][:], op_d[dy, t])
            return ops

        ops_embt2h = load_ops(op_embt2h_d, BF16, 32, 4, "oe2h")
        ops_embt2l = load_ops(op_embt2l_d, BF16, 32, 4, "oe2l")
        ops_embt1 = load_ops(op_embt1_d, BF16, 32, 4, "oe1")
        ops_c1t2 = load_ops(op_c1t2_d, F32, 32, 4, "oc12")
        ops_c1t1 = load_ops(op_c1t1_d, BF16, 32, 4, "oc11")
        ops_c2t2 = load_ops(op_c2t2_d, F32, 64, 4, "od12")
        ops_c2t1 = load_ops(op_c2t1_d, BF16, 64, 4, "od11")

        # ---------------- emb conv (64x32 8-tile, windowed DRAM input) ----
        # ops_list: one or two (hi, lo) bf16 operator sets; passes accumulate
        # in PSUM, so the hi/lo split reproduces the fp32 operator exactly.
        def emb_conv(oh_d, ops_list, dt, odt, bias, tags, opool=None,
                     wbufs=2, weng=None):
            sx = "f" if odt == F32 else "b"
            outs = [(opool or npool).tile([128, YB], odt, tag=tg, name=tg)
                    for tg in tags]
            with tc.tile_pool(name=f"ew{tags[0]}{sx}", bufs=wbufs) as wp, \
                 tc.tile_pool(name=f"ep{tags[0]}{sx}", bufs=2, space="PSUM") as pp:
                for q in range(4):
                    wins = []
                    for t in range(4):
                        w = wp.tile([128, 6, BL], dt, tag=f"w{t}", name=f"w{t}")
                        (weng or nc.gpsimd).dma_start(w[:], oh_d[t, q])
                        wins.append(w)
                    for yg in (2 * q, 2 * q + 1):
                        y0 = 2 * yg
                        ps = [pp.tile([128, 2 * BL], F32, tag=f"p{i}", name=f"p{i}")
                              for i in range(2)]
                        first = True
                        for dy in (0, -1, 1):
                            n0, N, ysrc = _clip_dy(y0, 2, dy)
                            ly = ysrc - (4 * q - 1)
                            nys = N // BL
                            for ops in ops_list:
                                for b in range(8):
                                    t, hh = b % 4, b // 4
                                    nc.tensor.matmul(
                                        ps[hh][32 * (b % 4):32 * (b % 4) + 32,
                                               n0:n0 + N],
                                        ops[dy + 1][t][hh * 64:hh * 64 + 56, :],
                                        wins[t][hh * 64:hh * 64 + 56,
                                                ly:ly + nys, :],
                                        start=first,
                                        stop=(dy == 1 and ops is ops_list[-1]),
                                        tile_position=(hh * 64, 32 * (b % 4)))
                                first = False
                        sl = slice(y0 * BL, (y0 + 2) * BL)
                        bb0 = bias[:] if bias is not None else 0.0
                        nc.scalar.activation(outs[0][:, sl], ps[0][:],
                                             AF.Identity, bias=bb0)
                        nc.scalar.activation(outs[1][:, sl], ps[1][:],
                                             AF.Identity, bias=bb0)
            return outs

        # -------- windowed x-pair builder: 2-chunk nat -> 4 win tensors ----
        def build_wins(nat2, dt, q, wp, wengs=None):
            ys, ye = max(0, 4 * q - 1), min(H, 4 * q + 5)
            ly0, ly1 = ys - (4 * q - 1), ye - (4 * q - 1)
            wins = []
            for t in range(4):
                w = wp.tile([128, 6, BL], dt, tag=f"w{t}", name=f"w{t}")
                wf = w.rearrange("p y i -> p (y i)")
                for hh in range(2):
                    b = t + 4 * hh
                    x0 = 2 * b - 1
                    if b == 0:
                        nc.vector.memset(w[0:64, :, :], 0.0)
                    if b == 7:
                        nc.vector.memset(w[64:128, :, :], 0.0)
                    xs_s, xs_e = max(0, x0), min(W, x0 + 4)
                    if xs_s < 8 < xs_e:
                        pieces = [(xs_s, 8), (8, xs_e)]
                    else:
                        pieces = [(xs_s, xs_e)]
                    for (a, bb) in pieces:
                        ch = a // 8
                        # flat contiguous (y,i) ranges on both sides: one
                        # big packet per partition instead of per-y rows
                        (wengs[(t + hh) % len(wengs)]
                         if wengs else nc.sync).dma_start(
                            wf[hh * 64 + (a - x0) * 16:hh * 64 + (bb - x0) * 16,
                               ly0 * BL:ly0 * BL + (ye - ys) * BL],
                            nat2[ch][(a % 8) * 16:(a % 8) * 16 + (bb - a) * 16,
                                     ys * BL:ye * BL])
                wins.append(w)
            return wins

        # ---------------- c1 conv (64x32 8-tile, windowed) -----------------
        def c1_conv(ins2, ops, dt, bias, tags, opool=None, wbufs=2,
                    wengs=None):
            sx = "f" if dt == F32 else "b"
            outs = [(opool or npool).tile([128, YB], dt, tag=tg, name=tg)
                    for tg in tags]
            with tc.tile_pool(name=f"cw{tags[0]}{sx}", bufs=wbufs) as wp, \
                 tc.tile_pool(name=f"cp{tags[0]}{sx}", bufs=2, space="PSUM") as pp:
                for q in range(4):
                    wins = build_wins(ins2, dt, q, wp, wengs)
                    for yg in (2 * q, 2 * q + 1):
                        y0 = 2 * yg
                        ps = [pp.tile([128, 2 * BL], F32, tag=f"p{i}", name=f"p{i}")
                              for i in range(2)]
                        first = True
                        for dy in (0, -1, 1):
                            n0, N, ysrc = _clip_dy(y0, 2, dy)
                            ly = ysrc - (4 * q - 1)
                            nys = N // BL
                            for b in range(8):
                                t, hh = b % 4, b // 4
                                nc.tensor.matmul(
                                    ps[hh][32 * (b % 4):32 * (b % 4) + 32,
                                           n0:n0 + N],
                                    ops[dy + 1][t][hh * 64:hh * 64 + 64, :],
                                    wins[t][hh * 64:hh * 64 + 64, ly:ly + nys, :],
                                    start=first, stop=(dy == 1),
                                    tile_position=(hh * 64, 32 * (b % 4)))
                            first = False
                        sl = slice(y0 * BL, (y0 + 2) * BL)
                        for i in range(2):
                            nc.scalar.activation(outs[i][:, sl], ps[i][:],
                                                 AF.Relu, bias=bias[:])
            return outs

        # ---------------- c2 conv (64x64 4-tile, windowed) -----------------
        # split16: evacuate exact relu as fp16 (hi, lo) pairs so the linear
        # can run 3-pass fp16 at bf16 rate while reconstructing fp32.
        def c2_conv(ins2, ops, dt, bias, tags, split16=False, lo_tags=None,
                    opool=None, wbufs=2, wengs=None):
            sx = "f" if dt == F32 else "b"
            odt = F16 if split16 else dt
            outs = [(opool or npool).tile([128, YB], odt, tag=tg, name=tg)
                    for tg in tags]
            los = ([(opool or npool).tile([128, YB], F16, tag=tg, name=tg)
                    for tg in lo_tags] if split16 else None)
            BORD = [0, 1, 4, 5, 2, 3, 6, 7]
            with tc.tile_pool(name=f"dw{tags[0]}{sx}", bufs=wbufs) as wp, \
                 tc.tile_pool(name=f"ds{tags[0]}{sx}", bufs=2) as sp, \
                 tc.tile_pool(name=f"dp{tags[0]}{sx}", bufs=2, space="PSUM") as pp:
                for q in range(4):
                    wins = build_wins(ins2, dt, q, wp, wengs)
                    for yg in (2 * q, 2 * q + 1):
                        y0 = 2 * yg
                        ps = [pp.tile([128, 2 * BL], F32, tag=f"p{i}", name=f"p{i}")
                              for i in range(4)]
                        first = True
                        for dy in (0, -1, 1):
                            n0, N, ysrc = _clip_dy(y0, 2, dy)
                            ly = ysrc - (4 * q - 1)
                            nys = N // BL
                            for b in BORD:
                                t, hh = b % 4, b // 4
                                nc.tensor.matmul(
                                    ps[b // 2][64 * (b % 2):64 * (b % 2) + 64,
                                               n0:n0 + N],
                                    ops[dy + 1][t][hh * 64:hh * 64 + 64, :],
                                    wins[t][hh * 64:hh * 64 + 64, ly:ly + nys, :],
                                    start=first, stop=(dy == 1),
                                    tile_position=(hh * 64, 64 * (b % 2)))
                            first = False
                        sl = slice(y0 * BL, (y0 + 2) * BL)
                        for i in range(4):
                            if split16:
                                sc32 = sp.tile([128, 2 * BL], F32, tag="sc",
                                               name="sc")
                                nc.scalar.activation(sc32[:], ps[i][:],
                                                     AF.Relu, bias=bias[:])
                                nc.vector.tensor_copy(outs[i][:, sl], sc32[:])
                                nc.gpsimd.tensor_sub(los[i][:, sl], sc32[:],
                                                     outs[i][:, sl])
                            else:
                                nc.scalar.activation(outs[i][:, sl], ps[i][:],
                                                     AF.Relu, bias=bias[:])
            return outs, los

        # ---------------- linear (M=img, N=E; returns (img, E) chunks) ----
        def linear(c2o, lw_d, dt, bias_row, tagp, dma_eng=None):
            embT = [epool.tile([128, ESZ], F32, tag=f"{tagp}T{m}", name=f"{tagp}T{m}")
                    for m in range(2)]
            with tc.tile_pool(name=f"lw{tagp}", bufs=8) as lwp, \
                 tc.tile_pool(name=f"lp{tagp}", bufs=1, space="PSUM") as pp:
                ps = [pp.tile([128, ESZ], F32, tag=f"p{m}", name=f"p{m}")
                      for m in range(2)]
                for k in range(64):
                    cch, y = k // 16, k % 16
                    lwt = lwp.tile([128, ESZ], dt, tag="lw", name="lw")
                    eng = dma_eng or (nc.scalar if k % 2 == 0 else nc.sync)
                    eng.dma_start(lwt[:], lw_d[k])
                    for m in range(2):
                        lhsT = c2o[cch][:, y * BL + 128 * m:y * BL + 128 * m + 128]
                        nc.tensor.matmul(ps[m][:], lhsT, lwt[:],
                                         start=(k == 0), stop=False)
                for m in range(2):
                    nc.tensor.matmul(ps[m][:], ones_k[:],
                                     bias_row[:], start=False, stop=True)
                for m in range(2):
                    nc.scalar.activation(embT[m][:], ps[m][:], AF.Identity)
            return embT

        # ----- fp16 hi/lo 3-pass linear: exact fp32 to ~2^-22 ------------
        def linear3(c2h, c2l, lwh_d, lwl_d, bias_row, tagp):
            embT = [epool.tile([128, ESZ], F32, tag=f"{tagp}T{m}", name=f"{tagp}T{m}")
                    for m in range(2)]
            with tc.tile_pool(name=f"lw{tagp}", bufs=8) as lwp, \
                 tc.tile_pool(name=f"lp{tagp}", bufs=1, space="PSUM") as pp:
                ps = [pp.tile([128, ESZ], F32, tag=f"p{m}", name=f"p{m}")
                      for m in range(2)]
                for k in range(64):
                    cch, y = k // 16, k % 16
                    lwh = lwp.tile([128, ESZ], F16, tag="lwh", name="lwh")
                    nc.scalar.dma_start(lwh[:], lwh_d[k])
                    lwl = lwp.tile([128, ESZ], F16, tag="lwl", name="lwl")
                    nc.sync.dma_start(lwl[:], lwl_d[k])
                    for m in range(2):
                        o = y * BL + 128 * m
                        hi = c2h[cch][:, o:o + 128]
                        lo = c2l[cch][:, o:o + 128]
                        nc.tensor.matmul(ps[m][:], hi, lwh[:],
                                         start=(k == 0), stop=False)
                        nc.tensor.matmul(ps[m][:], hi, lwl[:],
                                         start=False, stop=False)
                        nc.tensor.matmul(ps[m][:], lo, lwh[:],
                                         start=False, stop=False)
                for m in range(2):
                    nc.tensor.matmul(ps[m][:], ones_k[:],
                                     bias_row[:], start=False, stop=True)
                for m in range(2):
                    nc.scalar.activation(embT[m][:], ps[m][:], AF.Identity)
            return embT

        def transpose_back(embT, dt, tagp):
            """(img,E) 2 chunks -> (E,img) 4 chunks of dtype dt."""
            emb = [epool.tile([128, BL], dt, tag=f"{tagp}{e}", name=f"{tagp}{e}")
                   for e in range(4)]
            with tc.tile_pool(name=f"tp{tagp}", bufs=2, space="PSUM") as tpp:
                for m in range(2):
                    for e in range(4):
                        tp = tpp.tile([128, 128], F32, tag="tp", name="tp")
                        nc.tensor.transpose(tp[:], embT[m][:, 128 * e:128 * e + 128],
                                            ident_sb[:])
                        nc.vector.tensor_copy(emb[e][:, 128 * m:128 * m + 128], tp[:])
            return emb

        # ====== t2 tower: nested (LIFO) pool lifetimes ===================
        pse3 = ES.enter_context(tc.tile_pool(name="pse3", bufs=1))
        with tc.tile_pool(name="pc1f", bufs=1) as pc1f:
            with tc.tile_pool(name="pd3f", bufs=1) as pd3f:
                with nc.named_scope("t2emb"):
                    d3 = emb_conv(ohd_d, [ops_embt2h, ops_embt2l], BF16,
                                  F32, None, ["A0", "A1"], opool=pd3f)
                # t1emb here: its PE work + DRAM window loads fill the
                # seam while t2c1's windows build from d3
                with nc.named_scope("t1emb"):
                    se3 = emb_conv(ohs_d, [ops_embt1], BF16, BF16,
                                   bias_sb["b_se"], ["B0", "B1"],
                                   opool=pse3)
                with nc.named_scope("t2c1"):
                    c1o2 = c1_conv(d3, ops_c1t2, F32, bias_sb["b_c1t2"],
                                   ["B0", "B1"], opool=pc1f, wbufs=2)
            with tc.tile_pool(name="pf16", bufs=1) as pf16:
                with nc.named_scope("t2c2"):
                    c2h, c2l = c2_conv(c1o2, ops_c2t2, F32,
                                       bias_sb["b_c2t2"],
                                       ["H0", "H1", "H2", "H3"],
                                       split16=True,
                                       lo_tags=["L0", "L1", "L2", "L3"],
                                       opool=pf16, wbufs=2)
                with nc.named_scope("t2lin"):
                    embT2 = linear3(c2h, c2l, lw2h_d, lw2l_d,
                                    bl_sb["b_l2"], "e2")
                    embed2 = transpose_back(embT2, F32, "e2n")

        ES.enter_context(nc.named_scope("vq"))
        with tc.tile_pool(name="vq", bufs=1) as vqp, \
             tc.tile_pool(name="vqp", bufs=1, space="PSUM") as vpp:
            sps = [vpp.tile([128, NZ], F32, tag=f"s{m}", name=f"s{m}")
                   for m in range(2)]
            for e in range(4):
                for m in range(2):
                    nc.tensor.matmul(sps[m][:],
                                     embed2[e][:, 128 * m:128 * m + 128],
                                     znt_sb[e][:], start=(e == 0),
                                     stop=(e == 3))
            for m in range(2):
                sc = vqp.tile([128, NZ], F32, tag=f"sc{m}", name=f"sc{m}")
                nc.vector.tensor_copy(sc[:], sps[m][:])
                mx = vqp.tile([128, 8], F32, tag=f"mx{m}", name=f"mx{m}")
                nc.vector.max(mx[:], sc[:])
                ix = vqp.tile([128, 8], U32, tag=f"ix{m}", name=f"ix{m}")
                nc.vector.max_index(ix[:], mx[:], sc[:])
                ixf = vqp.tile([128, 1], F32, tag=f"ixf{m}", name=f"ixf{m}")
                nc.vector.tensor_copy(ixf[:], ix[:, :1])
                nc.sync.dma_start(iloc_d[128 * m:128 * m + 128], ixf[:])
            nc.gpsimd.collective_compute(
                "AllGather", mybir.AluOpType.bypass,
                replica_groups=[list(range(NCORES))],
                ins=[iloc_d[:]], outs=[ig_d[:]])
        npool = ES.enter_context(tc.tile_pool(name="nat", bufs=1))
        with nc.named_scope("t1c1"):
            c1o1 = c1_conv(se3, ops_c1t1, BF16, bias_sb["b_c1t1"],
                           ["C0", "C1"])

        zntb_sb = []
        for e in range(4):
            t = epool.tile([128, NZ], BF16, tag=f"zb{e}", name=f"zb{e}")
            nc.sync.dma_start(t[:], zntb_d[128 * e:128 * e + 128, :])
            zntb_sb.append(t)

        with nc.named_scope("t1c2"):
            c2o1, _ = c2_conv(c1o1, ops_c2t1, BF16, bias_sb["b_c2t1"],
                              ["A0", "A1", "B0", "B1"])

        # ================== t1 linear + norm-folded transpose =============
        with nc.named_scope("t1lin"):
            embT1 = linear(c2o1, lw1_d, BF16, bl_sb["b_l1"], "e1")

        with tc.tile_pool(name="nrm", bufs=1) as nrp:
            for m in range(2):
                sq = nrp.tile([128, ESZ], F32, tag="sq", name="sq")
                nc.vector.tensor_mul(sq[:], embT1[m][:], embT1[m][:])
                n2 = nrp.tile([128, 1], F32, tag="n2", name="n2")
                nc.vector.tensor_reduce(n2[:], sq[:], mybir.AxisListType.X,
                                        mybir.AluOpType.add)
                nc.scalar.sqrt(n2[:], n2[:])
                nc.vector.tensor_scalar_add(n2[:], n2[:], EPS)
                nc.vector.reciprocal(n2[:], n2[:])
                nc.vector.tensor_scalar_mul(n2[:], n2[:], esc)
                nc.vector.tensor_scalar_mul(embT1[m][:], embT1[m][:], n2[:])
        with nc.named_scope("t1tr"):
            e1b = transpose_back(embT1, BF16, "e1b")

        # ================== S1T = znT . e1norm  (z-part, img-cols) ========
        with nc.named_scope("fin"), \
             tc.tile_pool(name="fs1", bufs=2, space="PSUM") as fsp1, \
             tc.tile_pool(name="fs2", bufs=1, space="PSUM") as fsp2, \
             tc.tile_pool(name="fs3", bufs=2, space="PSUM") as fsp3, \
             tc.tile_pool(name="fo", bufs=4) as fop:
            s1t = []
            for zc in range(4):
                pss = fsp1.tile([128, BL], F32, tag="pss", name="pss")
                for e in range(4):
                    nc.tensor.matmul(pss[:],
                                     zntb_sb[e][:, 128 * zc:128 * zc + 128],
                                     e1b[e][:], start=(e == 0), stop=(e == 3))
                t = epool.tile([128, BL], BF16, tag=f"s1t{zc}", name=f"s1t{zc}")
                nc.vector.tensor_copy(t[:], pss[:])
                s1t.append(t)

            # ---- gather matrix G[zc][p, n] = (idx_global[n] == 128*zc+p) --
            figp_cm = tc.tile_pool(name="figp", bufs=1)
            figp = figp_cm.__enter__()
            igh = figp.tile([1, B], F16, tag="igh", name="igh")
            nc.gpsimd.dma_start(igh[:], ig_d.rearrange("a b -> b a"))
            psb = fsp2.tile([128, B], F32, tag="psb", name="psb")
            for h in range(4):
                nc.tensor.matmul(psb[:, 512 * h:512 * h + 512], ones_h[:],
                                 igh[:, 512 * h:512 * h + 512],
                                 start=True, stop=True)

            # ---- out = s1t.T @ G, G built per 512-col chunk on DVE --------
            with tc.tile_pool(name="gp", bufs=2) as gpool:
                for n in range(4):
                    gs = []
                    for zc in range(4):
                        g = gpool.tile([128, 512], BF16, tag=f"g{zc}",
                                       name=f"g{zc}")
                        nc.vector.tensor_scalar(
                            g[:], psb[:, 512 * n:512 * n + 512],
                            iotaz_sb[zc][:], None,
                            op0=mybir.AluOpType.is_equal)
                        gs.append(g)
                    for m in range(2):
                        fp = fsp3.tile([128, 512], F32, tag="f", name="f")
                        for zc in range(4):
                            nc.tensor.matmul(fp[:],
                                             s1t[zc][:, 128 * m:128 * m + 128],
                                             gs[zc][:],
                                             start=(zc == 0), stop=(zc == 3))
                        ob = fop.tile([128, 512], F32, tag="ob", name="ob")
                        nc.scalar.activation(ob[:], fp[:], AF.Identity)
                        (nc.sync if m == 0 else nc.scalar).dma_start(
                            out_d[128 * m:128 * m + 128,
                                  512 * n:512 * n + 512],
                            ob[:])
            figp_cm.__exit__(None, None, None)

    nc.compile()
    return nc


def make_in_maps(shared, percore):
    import ml_dtypes
    bf = ml_dtypes.bfloat16

    def b16(x):
        return np.asarray(x, np.float32).astype(bf)

    oph = b16(shared["op_emb"])
    opl = b16(shared["op_emb"].astype(np.float32) - oph.astype(np.float32))
    base = {
        "op_embt1": oph,
        "op_embt2h": oph,
        "op_embt2l": opl,
        "op_c1t1": b16(shared["op_c1t1"]),
        "op_c1t2": np.ascontiguousarray(shared["op_c1t2"], np.float32),
        "op_c2t1": b16(shared["op_c2t1"]),
        "op_c2t2": np.ascontiguousarray(shared["op_c2t2"], np.float32),
        "lw1": b16(shared["lw_t1"]),
        "lw2h": shared["lw_t2"].astype(np.float16),
        "lw2l": (shared["lw_t2"] -
                 shared["lw_t2"].astype(np.float16).astype(np.float32)
                 ).astype(np.float16),
        "b_se": shared["b_emb"], "b_c1t1": shared["b_c1t1"],
        "b_c1t2": shared["b_c1t2"], "b_c2t1": shared["b_c2t1"],
        "b_c2t2": shared["b_c2t2"],
        "b_l1": shared["b_l1"], "b_l2": shared["b_l2"],
        "znt": shared["znT"], "zntb": b16(shared["znT"]),
        "iotaz": shared["iotaz"],
        "ident": np.eye(128, dtype=np.float32),
    }
    maps = []
    for pc in percore:
        m = dict(base)
        m["ohs"] = np.ascontiguousarray(pc["ohs"].astype(bf))
        m["ohd"] = np.ascontiguousarray(pc["ohd"].astype(bf))
        maps.append(m)
    return maps


def kernel(**inputs):
    dsf = np.asarray(inputs.get("downscale_factor", 1)).reshape(-1)
    dsf = int(dsf[0]) if dsf.size else 1
    assert dsf == 1, f"only downscale_factor=1 supported, got {dsf}"
    shared, percore, esc = host_prep(inputs)
    nc = build_program(esc)
    maps = make_in_maps(shared, percore)
    res = run_bass_kernel_spmd(nc, maps, list(range(NCORES)))
    out = np.concatenate([res.results[c]["out"] for c in range(NCORES)],
                         axis=0)
    return out.astype(np.float32)


def run_for_test(inputs, trace=False):
    """test.py hook: returns (out, BassKernelResults)."""
    shared, percore, esc = host_prep(inputs)
    nc = build_program(esc)
    maps = make_in_maps(shared, percore)
    res = run_bass_kernel_spmd(nc, maps, list(range(NCORES)), trace=trace)
    out = np.concatenate([res.results[c]["out"] for c in range(NCORES)],
                         axis=0)
    return out.astype(np.float32), res


# revision 48
# speedup vs baseline: 1.0383x; 1.0383x over previous
"""TRN2 Bass kernel for nn_DSSMEmbed (vq_codebook).

Strategy (8 NeuronCores, data-parallel over batch, 256 imgs/core):
  - Activation layout: partitions = (x, channel) rows, free = (y, img).
  - 3x3 convs as Toeplitz matmuls over x-windows with batch streamed in N;
    dy handled by PSUM accumulation at shifted free-dim (y) offsets.
  - emb conv: 64x32 8-tile mode, windowed one-hot input from DRAM (K=56).
  - c1 conv:  64x32 8-tile, windowed y-pair buffers built by DMA.
  - c2 conv:  64x64 4-tile, windowed.
  - Tower1 (bf16) and tower2 (fp32, feeds VQ argmax exactly) phases are
    interleaved so each tower's window-build DMA hides under the other
    tower's PE work.
  - VQ: scores.T via PE (fp32), per-row max/max_index on DVE; only the
    ARGMAX INDICES are AllGathered (1KB), then each core builds a one-hot
    gather matrix G[z, n] = (idx_global[n] == z) on DVE and computes
    out = (znT_bf16 . e1norm).T @ G on the PE -- no codebook-row exchange,
    no z transposes.
  - embed1 norms via DVE square+reduce; 1/(|e|+eps) and exp(scale)
    folded into embT1 before its transpose, so the final product needs
    no post-scaling; output DMA'd per 512-column chunk as computed.
"""
import sys

sys.path.insert(0, "/opt/trn_rl_repo")

import numpy as np
import concourse.bass as bass
import concourse.bacc as bacc
import concourse.mybir as mybir
import concourse.tile as tile
from concourse.bass_utils import run_bass_kernel_spmd

F32 = mybir.dt.float32
F16 = mybir.dt.float16
BF16 = mybir.dt.bfloat16
U32 = mybir.dt.uint32
AF = mybir.ActivationFunctionType

NCORES = 8
B = 2048
BL = B // NCORES          # 256 imgs per core
H = W = 16
DICT, SE, CE, ESZ, NZ = 14, 8, 16, 512, 512
EPS = 1e-4
YB = H * BL               # free dim (y, img) = 4096

# ---------------------------------------------------------------------------
# host-side preprocessing
# ---------------------------------------------------------------------------


def make_windowed_oh(nat):
    """nat: (DICT, H, W, Bloc) one-hot -> (4, 4, 128, 6, Bloc).

    px=2: 8 blocks; tensor t holds block t at rows 0.. and block t+4 at
    rows 64..; rows w*14+d for window x' = 2b-1+w, w in 0..3.  Second dim
    is the y-quarter: quarter q covers global y in [4q-1, 4q+5) (clipped,
    duplicated halo) so each DMA load is contiguous per partition.
    """
    out = np.zeros((4, 4, 128, 6, nat.shape[-1]), dtype=np.int8)
    for b in range(8):
        t, h = b % 4, b // 4
        for w in range(4):
            xs = 2 * b - 1 + w
            if 0 <= xs < W:
                for q in range(4):
                    ys, ye = max(0, 4 * q - 1), min(H, 4 * q + 5)
                    out[t, q, h * 64 + w * DICT:h * 64 + (w + 1) * DICT,
                        ys - (4 * q - 1):ye - (4 * q - 1)] = nat[:, ys:ye, xs, :]
    return out


def op_emb_win(wfold):
    """Folded emb conv operator for 64x32 windowed scheme: (3, 4, 128, 32)."""
    op = np.zeros((3, 4, 128, 32), dtype=np.float32)
    for dy in range(3):
        blk = np.zeros((56, 32), dtype=np.float32)
        for w in range(4):
            for xr in range(2):
                dx = w - xr
                if 0 <= dx <= 2:
                    blk[w * DICT:(w + 1) * DICT, xr * 16:(xr + 1) * 16] = \
                        wfold[:, :, dy, dx].T
        for h in range(2):
            op[dy, :, h * 64:h * 64 + 56, :] = blk[None]
    return op


def op_conv_win(wc, c_in, c_out):
    """Windowed 64-row conv operator: (3, 4, 128, px*c_out) with px=2."""
    M = 2 * c_out
    op = np.zeros((3, 4, 128, M), dtype=np.float32)
    blk = np.zeros((4 * c_in, M), dtype=np.float32)
    for dy in range(3):
        blk[:] = 0.0
        for w in range(4):
            for xr in range(2):
                dx = w - xr
                if 0 <= dx <= 2:
                    blk[w * c_in:(w + 1) * c_in, xr * c_out:(xr + 1) * c_out] = \
                        wc[:, :, dy, dx].T
        for h in range(2):
            op[dy, :, h * 64:h * 64 + 4 * c_in, :] = blk[None]
        op[dy, 0, 0:c_in, :] = 0.0                    # b=0, w=0 (x'=-1)
        op[dy, 3, 64 + 3 * c_in:64 + 4 * c_in, :] = 0.0  # b=7, w=3 (x'=16)
    return op


def host_prep(inputs):
    s = np.asarray(inputs["s"])
    sp = np.asarray(inputs["s_prime"])
    se_w = np.asarray(inputs["state_embed"], dtype=np.float32)
    norms = np.sqrt((se_w * se_w).sum(1, keepdims=True))
    table = se_w / np.maximum(norms, 1.0)

    oh_s = (np.arange(DICT)[:, None, None, None] ==
            s.transpose(1, 2, 0)[None]).astype(np.float32)
    oh_sp = (np.arange(DICT)[:, None, None, None] ==
             sp.transpose(1, 2, 0)[None]).astype(np.float32)
    oh_d = oh_sp - oh_s

    emb_fold = np.einsum("oikl,di->odkl",
                         np.asarray(inputs["conv_embed_w"], np.float32), table)

    shared = {
        "op_emb": op_emb_win(emb_fold),
        "op_c1t1": op_conv_win(np.asarray(inputs["p1c1_w"], np.float32), 16, 16),
        "op_c1t2": op_conv_win(np.asarray(inputs["p2c1_w"], np.float32), 16, 16),
        "op_c2t1": op_conv_win(np.asarray(inputs["p1c2_w"], np.float32), 16, 32),
        "op_c2t2": op_conv_win(np.asarray(inputs["p2c2_w"], np.float32), 16, 32),
    }

    def reorder_lin(lw):
        # K order: (chunk c, y, row r), r = xr*32+ch, x = c*4+xr
        lw = np.asarray(lw, np.float32).reshape(ESZ, 32, H, W)
        lw = lw.transpose(3, 1, 2, 0).reshape(4, 4, 32, H, ESZ)  # (c,xr,ch,y,E)
        return np.ascontiguousarray(
            lw.transpose(0, 3, 1, 2, 4).reshape(4, H, 128, ESZ).reshape(64, 128, ESZ))

    shared["lw_t1"] = reorder_lin(inputs["p1l_w"])
    shared["lw_t2"] = reorder_lin(inputs["p2l_w"])

    zv = np.asarray(inputs["z_vectors"], np.float32)
    zn = zv / np.sqrt((zv * zv).sum(1, keepdims=True))
    shared["znT"] = np.ascontiguousarray(zn.T)

    def conv_bias(bvec, c_out):
        reps = 128 // c_out
        return np.ascontiguousarray(
            np.tile(np.asarray(bvec, np.float32), reps)[:, None])

    shared["b_emb"] = conv_bias(inputs["conv_embed_b"], 16)
    shared["b_c1t1"] = conv_bias(inputs["p1c1_b"], 16)
    shared["b_c1t2"] = conv_bias(inputs["p2c1_b"], 16)
    shared["b_c2t1"] = conv_bias(inputs["p1c2_b"], 32)
    shared["b_c2t2"] = conv_bias(inputs["p2c2_b"], 32)
    shared["b_l1"] = np.ascontiguousarray(
        np.asarray(inputs["p1l_b"], np.float32).reshape(1, ESZ))
    shared["b_l2"] = np.ascontiguousarray(
        np.asarray(inputs["p2l_b"], np.float32).reshape(1, ESZ))

    # per-partition iota for the one-hot gather build: iotaz[zc][p] = 128*zc+p
    shared["iotaz"] = np.ascontiguousarray(
        (np.arange(NZ, dtype=np.float32).reshape(4, 128, 1)))

    esc = float(np.exp(np.asarray(inputs["scale"], np.float32).reshape(-1)[0]))

    percore = []
    for c in range(NCORES):
        sl = slice(c * BL, (c + 1) * BL)
        percore.append({
            "ohs": make_windowed_oh(oh_s[..., sl]),
            "ohd": make_windowed_oh(oh_d[..., sl]),
        })
    return shared, percore, esc


# ---------------------------------------------------------------------------
# device program
# ---------------------------------------------------------------------------


def _clip_dy(y0, ny, dy):
    s = max(y0, -dy)
    e = min(y0 + ny, H - dy)
    if s >= e:
        return None
    return (s - y0) * BL, (e - s) * BL, s + dy


def build_program(esc, debug=False):
    from contextlib import ExitStack
    nc = bacc.Bacc("TRN2", target_bir_lowering=False, debug=False,
                   num_devices=NCORES)

    def din(name, shape, dt):
        return nc.dram_tensor(name, list(shape), dt, kind="ExternalInput").ap()

    ohs_d = din("ohs", (4, 4, 128, 6, BL), BF16)
    ohd_d = din("ohd", (4, 4, 128, 6, BL), BF16)
    op_embt1_d = din("op_embt1", (3, 4, 128, 32), BF16)
    op_embt2h_d = din("op_embt2h", (3, 4, 128, 32), BF16)
    op_embt2l_d = din("op_embt2l", (3, 4, 128, 32), BF16)
    op_c1t1_d = din("op_c1t1", (3, 4, 128, 32), BF16)
    op_c1t2_d = din("op_c1t2", (3, 4, 128, 32), F32)
    op_c2t1_d = din("op_c2t1", (3, 4, 128, 64), BF16)
    op_c2t2_d = din("op_c2t2", (3, 4, 128, 64), F32)
    lw1_d = din("lw1", (64, 128, ESZ), BF16)
    lw2h_d = din("lw2h", (64, 128, ESZ), F16)
    lw2l_d = din("lw2l", (64, 128, ESZ), F16)
    b_se_d = din("b_se", (128, 1), F32)
    b_c1t1_d = din("b_c1t1", (128, 1), F32)
    b_c1t2_d = din("b_c1t2", (128, 1), F32)
    b_c2t1_d = din("b_c2t1", (128, 1), F32)
    b_c2t2_d = din("b_c2t2", (128, 1), F32)
    b_l1_d = din("b_l1", (1, ESZ), F32)
    b_l2_d = din("b_l2", (1, ESZ), F32)
    znt_d = din("znt", (ESZ, NZ), F32)
    zntb_d = din("zntb", (ESZ, NZ), BF16)
    iotaz_d = din("iotaz", (4, 128, 1), F32)
    ident_d = din("ident", (128, 128), F32)

    out_d = nc.dram_tensor("out", [BL, B], F32, kind="ExternalOutput").ap()

    iloc_d = nc.dram_tensor("iloc", [BL, 1], F32).ap()
    ig_d = nc.dram_tensor("ig", [NCORES * BL, 1], F32,
                          addr_space="Shared").ap()

    with tile.TileContext(nc) as tc, ExitStack() as ES:
        cst = ES.enter_context(tc.tile_pool(name="cst", bufs=1))
        epool = ES.enter_context(tc.tile_pool(name="emb", bufs=1))
        npool = None

        ident_sb = cst.tile([128, 128], F32, tag="ident", name="ident")
        nc.sync.dma_start(ident_sb[:], ident_d[:])
        bias_sb = {}
        for nm, d in [("b_se", b_se_d), ("b_c1t1", b_c1t1_d),
                      ("b_c1t2", b_c1t2_d), ("b_c2t1", b_c2t1_d),
                      ("b_c2t2", b_c2t2_d)]:
            t = cst.tile([128, 1], F32, tag=nm, name=nm)
            nc.sync.dma_start(t[:], d[:])
            bias_sb[nm] = t
        bl_sb = {}
        for nm, d in [("b_l1", b_l1_d), ("b_l2", b_l2_d)]:
            t = cst.tile([1, ESZ], F32, tag=f"{nm}r", name=f"{nm}r")
            nc.sync.dma_start(t[:], d[:])
            bl_sb[nm] = t
        ones_k = cst.tile([1, 128], F32, tag="ones_k", name="ones_k")
        nc.vector.memset(ones_k[:], 1.0)
        ones_h = cst.tile([1, 128], F16, tag="ones_h", name="ones_h")
        nc.vector.memset(ones_h[:], 1.0)
        znt_sb = []
        for e in range(4):
            t = cst.tile([128, NZ], F32, tag=f"znt{e}", name=f"znt{e}")
            nc.scalar.dma_start(t[:], znt_d[128 * e:128 * e + 128, :])
            znt_sb.append(t)
        iotaz_sb = []
        for zc in range(4):
            t = cst.tile([128, 1], F32, tag=f"iota{zc}", name=f"iota{zc}")
            nc.sync.dma_start(t[:], iotaz_d[zc])
            iotaz_sb.append(t)

        def load_ops(op_d, dt, width, nt, pfx):
            ops = [[cst.tile([128, width], dt, tag=f"{pfx}{dy}{t}",
                             name=f"{pfx}{dy}{t}") for t in range(nt)]
                   for dy in range(3)]
            for dy in range(3):
                for t in range(nt):
                    nc.sync.dma_start(ops[dy][t][:], op_d[dy, t])
            return ops

        ops_embt2h = load_ops(op_embt2h_d, BF16, 32, 4, "oe2h")
        ops_embt2l = load_ops(op_embt2l_d, BF16, 32, 4, "oe2l")
        ops_embt1 = load_ops(op_embt1_d, BF16, 32, 4, "oe1")
        ops_c1t2 = load_ops(op_c1t2_d, F32, 32, 4, "oc12")
        ops_c1t1 = load_ops(op_c1t1_d, BF16, 32, 4, "oc11")
        ops_c2t2 = load_ops(op_c2t2_d, F32, 64, 4, "od12")
        ops_c2t1 = load_ops(op_c2t1_d, BF16, 64, 4, "od11")

        # ---------------- emb conv (64x32 8-tile, windowed DRAM input) ----
        # ops_list: one or two (hi, lo) bf16 operator sets; passes accumulate
        # in PSUM, so the hi/lo split reproduces the fp32 operator exactly.
        def emb_conv(oh_d, ops_list, dt, odt, bias, tags, opool=None,
                     wbufs=2, weng=None):
            sx = "f" if odt == F32 else "b"
            outs = [(opool or npool).tile([128, YB], odt, tag=tg, name=tg)
                    for tg in tags]
            with tc.tile_pool(name=f"ew{tags[0]}{sx}", bufs=wbufs) as wp, \
                 tc.tile_pool(name=f"ep{tags[0]}{sx}", bufs=2, space="PSUM") as pp:
                for q in range(4):
                    wins = []
                    for t in range(4):
                        w = wp.tile([128, 6, BL], dt, tag=f"w{t}", name=f"w{t}")
                        (weng or nc.gpsimd).dma_start(w[:], oh_d[t, q])
                        wins.append(w)
                    for yg in (2 * q, 2 * q + 1):
                        y0 = 2 * yg
                        ps = [pp.tile([128, 2 * BL], F32, tag=f"p{i}", name=f"p{i}")
                              for i in range(2)]
                        first = True
                        for dy in (0, -1, 1):
                            n0, N, ysrc = _clip_dy(y0, 2, dy)
                            ly = ysrc - (4 * q - 1)
                            nys = N // BL
                            for ops in ops_list:
                                for b in range(8):
                                    t, hh = b % 4, b // 4
                                    nc.tensor.matmul(
                                        ps[hh][32 * (b % 4):32 * (b % 4) + 32,
                                               n0:n0 + N],
                                        ops[dy + 1][t][hh * 64:hh * 64 + 56, :],
                                        wins[t][hh * 64:hh * 64 + 56,
                                                ly:ly + nys, :],
                                        start=first,
                                        stop=(dy == 1 and ops is ops_list[-1]),
                                        tile_position=(hh * 64, 32 * (b % 4)))
                                first = False
                        sl = slice(y0 * BL, (y0 + 2) * BL)
                        bb0 = bias[:] if bias is not None else 0.0
                        nc.scalar.activation(outs[0][:, sl], ps[0][:],
                                             AF.Identity, bias=bb0)
                        nc.scalar.activation(outs[1][:, sl], ps[1][:],
                                             AF.Identity, bias=bb0)
            return outs

        # -------- windowed x-pair builder: 2-chunk nat -> 4 win tensors ----
        def build_wins(nat2, dt, q, wp, wengs=None):
            ys, ye = max(0, 4 * q - 1), min(H, 4 * q + 5)
            ly0, ly1 = ys - (4 * q - 1), ye - (4 * q - 1)
            wins = []
            for t in range(4):
                w = wp.tile([128, 6, BL], dt, tag=f"w{t}", name=f"w{t}")
                wf = w.rearrange("p y i -> p (y i)")
                for hh in range(2):
                    b = t + 4 * hh
                    x0 = 2 * b - 1
                    if b == 0:
                        nc.vector.memset(w[0:64, :, :], 0.0)
                    if b == 7:
                        nc.vector.memset(w[64:128, :, :], 0.0)
                    xs_s, xs_e = max(0, x0), min(W, x0 + 4)
                    if xs_s < 8 < xs_e:
                        pieces = [(xs_s, 8), (8, xs_e)]
                    else:
                        pieces = [(xs_s, xs_e)]
                    for (a, bb) in pieces:
                        ch = a // 8
                        # flat contiguous (y,i) ranges on both sides: one
                        # big packet per partition instead of per-y rows
                        (wengs[(t + hh) % len(wengs)]
                         if wengs else nc.sync).dma_start(
                            wf[hh * 64 + (a - x0) * 16:hh * 64 + (bb - x0) * 16,
                               ly0 * BL:ly0 * BL + (ye - ys) * BL],
                            nat2[ch][(a % 8) * 16:(a % 8) * 16 + (bb - a) * 16,
                                     ys * BL:ye * BL])
                wins.append(w)
            return wins

        # ---------------- c1 conv (64x32 8-tile, windowed) -----------------
        def c1_conv(ins2, ops, dt, bias, tags, opool=None, wbufs=2,
                    wengs=None):
            sx = "f" if dt == F32 else "b"
            outs = [(opool or npool).tile([128, YB], dt, tag=tg, name=tg)
                    for tg in tags]
            with tc.tile_pool(name=f"cw{tags[0]}{sx}", bufs=wbufs) as wp, \
                 tc.tile_pool(name=f"cp{tags[0]}{sx}", bufs=2, space="PSUM") as pp:
                for q in range(4):
                    wins = build_wins(ins2, dt, q, wp, wengs)
                    for yg in (2 * q, 2 * q + 1):
                        y0 = 2 * yg
                        ps = [pp.tile([128, 2 * BL], F32, tag=f"p{i}", name=f"p{i}")
                              for i in range(2)]
                        first = True
                        for dy in (0, -1, 1):
                            n0, N, ysrc = _clip_dy(y0, 2, dy)
                            ly = ysrc - (4 * q - 1)
                            nys = N // BL
                            for b in range(8):
                                t, hh = b % 4, b // 4
                                nc.tensor.matmul(
                                    ps[hh][32 * (b % 4):32 * (b % 4) + 32,
                                           n0:n0 + N],
                                    ops[dy + 1][t][hh * 64:hh * 64 + 64, :],
                                    wins[t][hh * 64:hh * 64 + 64, ly:ly + nys, :],
                                    start=first, stop=(dy == 1),
                                    tile_position=(hh * 64, 32 * (b % 4)))
                            first = False
                        sl = slice(y0 * BL, (y0 + 2) * BL)
                        for i in range(2):
                            nc.scalar.activation(outs[i][:, sl], ps[i][:],
                                                 AF.Relu, bias=bias[:])
            return outs

        # ---------------- c2 conv (64x64 4-tile, windowed) -----------------
        # split16: evacuate exact relu as fp16 (hi, lo) pairs so the linear
        # can run 3-pass fp16 at bf16 rate while reconstructing fp32.
        def c2_conv(ins2, ops, dt, bias, tags, split16=False, lo_tags=None,
                    opool=None, wbufs=2, wengs=None):
            sx = "f" if dt == F32 else "b"
            odt = F16 if split16 else dt
            outs = [(opool or npool).tile([128, YB], odt, tag=tg, name=tg)
                    for tg in tags]
            los = ([(opool or npool).tile([128, YB], F16, tag=tg, name=tg)
                    for tg in lo_tags] if split16 else None)
            BORD = [0, 1, 4, 5, 2, 3, 6, 7]
            with tc.tile_pool(name=f"dw{tags[0]}{sx}", bufs=wbufs) as wp, \
                 tc.tile_pool(name=f"ds{tags[0]}{sx}", bufs=2) as sp, \
                 tc.tile_pool(name=f"dp{tags[0]}{sx}", bufs=2, space="PSUM") as pp:
                for q in range(4):
                    wins = build_wins(ins2, dt, q, wp, wengs)
                    for yg in (2 * q, 2 * q + 1):
                        y0 = 2 * yg
                        ps = [pp.tile([128, 2 * BL], F32, tag=f"p{i}", name=f"p{i}")
                              for i in range(4)]
                        first = True
                        for dy in (0, -1, 1):
                            n0, N, ysrc = _clip_dy(y0, 2, dy)
                            ly = ysrc - (4 * q - 1)
                            nys = N // BL
                            for b in BORD:
                                t, hh = b % 4, b // 4
                                nc.tensor.matmul(
                                    ps[b // 2][64 * (b % 2):64 * (b % 2) + 64,
                                               n0:n0 + N],
                                    ops[dy + 1][t][hh * 64:hh * 64 + 64, :],
                                    wins[t][hh * 64:hh * 64 + 64, ly:ly + nys, :],
                                    start=first, stop=(dy == 1),
                                    tile_position=(hh * 64, 64 * (b % 2)))
                            first = False
                        sl = slice(y0 * BL, (y0 + 2) * BL)
                        for i in range(4):
                            if split16:
                                sc32 = sp.tile([128, 2 * BL], F32, tag="sc",
                                               name="sc")
                                nc.scalar.activation(sc32[:], ps[i][:],
                                                     AF.Relu, bias=bias[:])
                                nc.vector.tensor_copy(outs[i][:, sl], sc32[:])
                                nc.gpsimd.tensor_sub(los[i][:, sl], sc32[:],
                                                     outs[i][:, sl])
                            else:
                                nc.scalar.activation(outs[i][:, sl], ps[i][:],
                                                     AF.Relu, bias=bias[:])
            return outs, los

        # ---------------- linear (M=img, N=E; returns (img, E) chunks) ----
        def linear(c2o, lw_d, dt, bias_row, tagp, dma_eng=None):
            embT = [epool.tile([128, ESZ], F32, tag=f"{tagp}T{m}", name=f"{tagp}T{m}")
                    for m in range(2)]
            with tc.tile_pool(name=f"lw{tagp}", bufs=8) as lwp, \
                 tc.tile_pool(name=f"lp{tagp}", bufs=1, space="PSUM") as pp:
                ps = [pp.tile([128, ESZ], F32, tag=f"p{m}", name=f"p{m}")
                      for m in range(2)]
                for k in range(64):
                    cch, y = k // 16, k % 16
                    lwt = lwp.tile([128, ESZ], dt, tag="lw", name="lw")
                    # scalar-only: the sync half would queue behind t1c2's
                    # window builds (head-of-line), stalling half the tiles
                    (dma_eng or nc.scalar).dma_start(lwt[:], lw_d[k])
                    for m in range(2):
                        lhsT = c2o[cch][:, y * BL + 128 * m:y * BL + 128 * m + 128]
                        nc.tensor.matmul(ps[m][:], lhsT, lwt[:],
                                         start=(k == 0), stop=False)
                for m in range(2):
                    nc.tensor.matmul(ps[m][:], ones_k[:],
                                     bias_row[:], start=False, stop=True)
                for m in range(2):
                    nc.scalar.activation(embT[m][:], ps[m][:], AF.Identity)
            return embT

        # ----- fp16 hi/lo 3-pass linear: exact fp32 to ~2^-22 ------------
        def linear3(c2h, c2l, lwh_d, lwl_d, bias_row, tagp):
            embT = [epool.tile([128, ESZ], F32, tag=f"{tagp}T{m}", name=f"{tagp}T{m}")
                    for m in range(2)]
            with tc.tile_pool(name=f"lw{tagp}", bufs=8) as lwp, \
                 tc.tile_pool(name=f"lp{tagp}", bufs=1, space="PSUM") as pp:
                ps = [pp.tile([128, ESZ], F32, tag=f"p{m}", name=f"p{m}")
                      for m in range(2)]
                for k in range(64):
                    cch, y = k // 16, k % 16
                    lwh = lwp.tile([128, ESZ], F16, tag="lwh", name="lwh")
                    nc.scalar.dma_start(lwh[:], lwh_d[k])
                    lwl = lwp.tile([128, ESZ], F16, tag="lwl", name="lwl")
                    nc.sync.dma_start(lwl[:], lwl_d[k])
                    for m in range(2):
                        o = y * BL + 128 * m
                        hi = c2h[cch][:, o:o + 128]
                        lo = c2l[cch][:, o:o + 128]
                        nc.tensor.matmul(ps[m][:], hi, lwh[:],
                                         start=(k == 0), stop=False)
                        nc.tensor.matmul(ps[m][:], hi, lwl[:],
                                         start=False, stop=False)
                        nc.tensor.matmul(ps[m][:], lo, lwh[:],
                                         start=False, stop=False)
                for m in range(2):
                    nc.tensor.matmul(ps[m][:], ones_k[:],
                                     bias_row[:], start=False, stop=True)
                for m in range(2):
                    nc.scalar.activation(embT[m][:], ps[m][:], AF.Identity)
            return embT

        def transpose_back(embT, dt, tagp):
            """(img,E) 2 chunks -> (E,img) 4 chunks of dtype dt."""
            emb = [epool.tile([128, BL], dt, tag=f"{tagp}{e}", name=f"{tagp}{e}")
                   for e in range(4)]
            with tc.tile_pool(name=f"tp{tagp}", bufs=2, space="PSUM") as tpp:
                for m in range(2):
                    for e in range(4):
                        tp = tpp.tile([128, 128], F32, tag="tp", name="tp")
                        nc.tensor.transpose(tp[:], embT[m][:, 128 * e:128 * e + 128],
                                            ident_sb[:])
                        nc.vector.tensor_copy(emb[e][:, 128 * m:128 * m + 128], tp[:])
            return emb

        # ====== t2 tower: nested (LIFO) pool lifetimes ===================
        pse3 = ES.enter_context(tc.tile_pool(name="pse3", bufs=1))
        with tc.tile_pool(name="pc1f", bufs=1) as pc1f:
            with tc.tile_pool(name="pd3f", bufs=1) as pd3f:
                with nc.named_scope("t2emb"):
                    d3 = emb_conv(ohd_d, [ops_embt2h, ops_embt2l], BF16,
                                  F32, None, ["A0", "A1"], opool=pd3f)
                # t1emb here: its PE work + DRAM window loads fill the
                # seam while t2c1's windows build from d3
                with nc.named_scope("t1emb"):
                    se3 = emb_conv(ohs_d, [ops_embt1], BF16, BF16,
                                   bias_sb["b_se"], ["B0", "B1"],
                                   opool=pse3)
                with nc.named_scope("t2c1"):
                    c1o2 = c1_conv(d3, ops_c1t2, F32, bias_sb["b_c1t2"],
                                   ["B0", "B1"], opool=pc1f, wbufs=2)
            with tc.tile_pool(name="pf16", bufs=1) as pf16:
                with nc.named_scope("t2c2"):
                    c2h, c2l = c2_conv(c1o2, ops_c2t2, F32,
                                       bias_sb["b_c2t2"],
                                       ["H0", "H1", "H2", "H3"],
                                       split16=True,
                                       lo_tags=["L0", "L1", "L2", "L3"],
                                       opool=pf16, wbufs=2)
                with nc.named_scope("t2lin"):
                    embT2 = linear3(c2h, c2l, lw2h_d, lw2l_d,
                                    bl_sb["b_l2"], "e2")
                    embed2 = transpose_back(embT2, F32, "e2n")

        ES.enter_context(nc.named_scope("vq"))
        with tc.tile_pool(name="vq", bufs=1) as vqp, \
             tc.tile_pool(name="vqp", bufs=1, space="PSUM") as vpp:
            sps = [vpp.tile([128, NZ], F32, tag=f"s{m}", name=f"s{m}")
                   for m in range(2)]
            for e in range(4):
                for m in range(2):
                    nc.tensor.matmul(sps[m][:],
                                     embed2[e][:, 128 * m:128 * m + 128],
                                     znt_sb[e][:], start=(e == 0),
                                     stop=(e == 3))
            for m in range(2):
                sc = vqp.tile([128, NZ], F32, tag=f"sc{m}", name=f"sc{m}")
                nc.vector.tensor_copy(sc[:], sps[m][:])
                mx = vqp.tile([128, 8], F32, tag=f"mx{m}", name=f"mx{m}")
                nc.vector.max(mx[:], sc[:])
                ix = vqp.tile([128, 8], U32, tag=f"ix{m}", name=f"ix{m}")
                nc.vector.max_index(ix[:], mx[:], sc[:])
                ixf = vqp.tile([128, 1], F32, tag=f"ixf{m}", name=f"ixf{m}")
                nc.vector.tensor_copy(ixf[:], ix[:, :1])
                nc.sync.dma_start(iloc_d[128 * m:128 * m + 128], ixf[:])
            nc.gpsimd.collective_compute(
                "AllGather", mybir.AluOpType.bypass,
                replica_groups=[list(range(NCORES))],
                ins=[iloc_d[:]], outs=[ig_d[:]])
        npool = ES.enter_context(tc.tile_pool(name="nat", bufs=1))
        with nc.named_scope("t1c1"):
            c1o1 = c1_conv(se3, ops_c1t1, BF16, bias_sb["b_c1t1"],
                           ["C0", "C1"])

        zntb_sb = []
        for e in range(4):
            t = epool.tile([128, NZ], BF16, tag=f"zb{e}", name=f"zb{e}")
            nc.sync.dma_start(t[:], zntb_d[128 * e:128 * e + 128, :])
            zntb_sb.append(t)

        with nc.named_scope("t1c2"):
            c2o1, _ = c2_conv(c1o1, ops_c2t1, BF16, bias_sb["b_c2t1"],
                              ["A0", "A1", "B0", "B1"])

        # ================== t1 linear + norm-folded transpose =============
        with nc.named_scope("t1lin"):
            embT1 = linear(c2o1, lw1_d, BF16, bl_sb["b_l1"], "e1")

        with tc.tile_pool(name="nrm", bufs=1) as nrp:
            for m in range(2):
                sq = nrp.tile([128, ESZ], F32, tag="sq", name="sq")
                nc.vector.tensor_mul(sq[:], embT1[m][:], embT1[m][:])
                n2 = nrp.tile([128, 1], F32, tag="n2", name="n2")
                nc.vector.tensor_reduce(n2[:], sq[:], mybir.AxisListType.X,
                                        mybir.AluOpType.add)
                nc.scalar.sqrt(n2[:], n2[:])
                nc.vector.tensor_scalar_add(n2[:], n2[:], EPS)
                nc.vector.reciprocal(n2[:], n2[:])
                nc.vector.tensor_scalar_mul(n2[:], n2[:], esc)
                nc.vector.tensor_scalar_mul(embT1[m][:], embT1[m][:], n2[:])
        with nc.named_scope("t1tr"):
            e1b = transpose_back(embT1, BF16, "e1b")

        # ================== S1T = znT . e1norm  (z-part, img-cols) ========
        with nc.named_scope("fin"), \
             tc.tile_pool(name="fs1", bufs=2, space="PSUM") as fsp1, \
             tc.tile_pool(name="fs2", bufs=1, space="PSUM") as fsp2, \
             tc.tile_pool(name="fs3", bufs=2, space="PSUM") as fsp3, \
             tc.tile_pool(name="fo", bufs=4) as fop:
            s1t = []
            for zc in range(4):
                pss = fsp1.tile([128, BL], F32, tag="pss", name="pss")
                for e in range(4):
                    nc.tensor.matmul(pss[:],
                                     zntb_sb[e][:, 128 * zc:128 * zc + 128],
                                     e1b[e][:], start=(e == 0), stop=(e == 3))
                t = epool.tile([128, BL], BF16, tag=f"s1t{zc}", name=f"s1t{zc}")
                nc.vector.tensor_copy(t[:], pss[:])
                s1t.append(t)

            # ---- gather matrix G[zc][p, n] = (idx_global[n] == 128*zc+p) --
            figp_cm = tc.tile_pool(name="figp", bufs=1)
            figp = figp_cm.__enter__()
            igh = figp.tile([1, B], F16, tag="igh", name="igh")
            nc.gpsimd.dma_start(igh[:], ig_d.rearrange("a b -> b a"))
            psb = fsp2.tile([128, B], F32, tag="psb", name="psb")
            for h in range(4):
                nc.tensor.matmul(psb[:, 512 * h:512 * h + 512], ones_h[:],
                                 igh[:, 512 * h:512 * h + 512],
                                 start=True, stop=True)

            # ---- out = s1t.T @ G, G built per 512-col chunk on DVE --------
            with tc.tile_pool(name="gp", bufs=2) as gpool:
                for n in range(4):
                    gs = []
                    for zc in range(4):
                        g = gpool.tile([128, 512], BF16, tag=f"g{zc}",
                                       name=f"g{zc}")
                        nc.vector.tensor_scalar(
                            g[:], psb[:, 512 * n:512 * n + 512],
                            iotaz_sb[zc][:], None,
                            op0=mybir.AluOpType.is_equal)
                        gs.append(g)
                    for m in range(2):
                        fp = fsp3.tile([128, 512], F32, tag="f", name="f")
                        for zc in range(4):
                            nc.tensor.matmul(fp[:],
                                             s1t[zc][:, 128 * m:128 * m + 128],
                                             gs[zc][:],
                                             start=(zc == 0), stop=(zc == 3))
                        ob = fop.tile([128, 512], F32, tag="ob", name="ob")
                        nc.scalar.activation(ob[:], fp[:], AF.Identity)
                        (nc.sync if m == 0 else nc.scalar).dma_start(
                            out_d[128 * m:128 * m + 128,
                                  512 * n:512 * n + 512],
                            ob[:])
            figp_cm.__exit__(None, None, None)

    nc.compile()
    return nc


def make_in_maps(shared, percore):
    import ml_dtypes
    bf = ml_dtypes.bfloat16

    def b16(x):
        return np.asarray(x, np.float32).astype(bf)

    oph = b16(shared["op_emb"])
    opl = b16(shared["op_emb"].astype(np.float32) - oph.astype(np.float32))
    base = {
        "op_embt1": oph,
        "op_embt2h": oph,
        "op_embt2l": opl,
        "op_c1t1": b16(shared["op_c1t1"]),
        "op_c1t2": np.ascontiguousarray(shared["op_c1t2"], np.float32),
        "op_c2t1": b16(shared["op_c2t1"]),
        "op_c2t2": np.ascontiguousarray(shared["op_c2t2"], np.float32),
        "lw1": b16(shared["lw_t1"]),
        "lw2h": shared["lw_t2"].astype(np.float16),
        "lw2l": (shared["lw_t2"] -
                 shared["lw_t2"].astype(np.float16).astype(np.float32)
                 ).astype(np.float16),
        "b_se": shared["b_emb"], "b_c1t1": shared["b_c1t1"],
        "b_c1t2": shared["b_c1t2"], "b_c2t1": shared["b_c2t1"],
        "b_c2t2": shared["b_c2t2"],
        "b_l1": shared["b_l1"], "b_l2": shared["b_l2"],
        "znt": shared["znT"], "zntb": b16(shared["znT"]),
        "iotaz": shared["iotaz"],
        "ident": np.eye(128, dtype=np.float32),
    }
    maps = []
    for pc in percore:
        m = dict(base)
        m["ohs"] = np.ascontiguousarray(pc["ohs"].astype(bf))
        m["ohd"] = np.ascontiguousarray(pc["ohd"].astype(bf))
        maps.append(m)
    return maps


def kernel(**inputs):
    dsf = np.asarray(inputs.get("downscale_factor", 1)).reshape(-1)
    dsf = int(dsf[0]) if dsf.size else 1
    assert dsf == 1, f"only downscale_factor=1 supported, got {dsf}"
    shared, percore, esc = host_prep(inputs)
    nc = build_program(esc)
    maps = make_in_maps(shared, percore)
    res = run_bass_kernel_spmd(nc, maps, list(range(NCORES)))
    out = np.concatenate([res.results[c]["out"] for c in range(NCORES)],
                         axis=0)
    return out.astype(np.float32)


def run_for_test(inputs, trace=False):
    """test.py hook: returns (out, BassKernelResults)."""
    shared, percore, esc = host_prep(inputs)
    nc = build_program(esc)
    maps = make_in_maps(shared, percore)
    res = run_bass_kernel_spmd(nc, maps, list(range(NCORES)), trace=trace)
    out = np.concatenate([res.results[c]["out"] for c in range(NCORES)],
                         axis=0)
    return out.astype(np.float32), res
